# revision 34
# baseline (speedup 1.0000x reference)
"""KBertGATEnricher Trainium2 kernel.

Sharding: data-parallel over batch (8 batches -> 8 cores) for the whole
model; each core computes its 256 tokens against the FULL vocab, so there
are no collectives at all. The 31.5MB f16 vocab weight streams in
1024-column chunk pairs on the gpsimd DMA queue while the PE runs the
output GEMM out of 8 PSUM banks.

log_softmax without a second full exp pass: with q = elu(z)+1 =
max(z,0) + t and t = min(e^z,1),

    S = sum(e^z) - n_pad + sum(exp(t-1) - t)

sum(e^z) rides the f32 accumulators of the exp pass we need anyway for
elu (bf16 output avoids both f16 overflow at z~21 and f16-subnormal
flushing near z~0); exp(t-1)-t is bounded in [0, 1/e] and vanishes for
z>=0, so its sum is estimated from a 2048-column sample (~0.3% error on S
against a 2e-2 budget). lnS is therefore ready ~2us after the last GEMM
chunk and the tail is just the final q-1-lnS pass (4x-mode DVE) overlapped
with the 15.7MB output DMA.

Self-contained: hardcodes all shapes; only imports the system-installed
concourse runtime.
"""

import os
import sys

sys.path.insert(0, "/opt/trn_rl_repo")

import numpy as np

from concourse import bass, bacc, mybir, tile
from concourse.bass_utils import run_bass_kernel_spmd

F32 = mybir.dt.float32
F16 = mybir.dt.float16
BF16 = mybir.dt.bfloat16

B, N, D, H, F, V = 8, 256, 768, 4, 128, 30522
NCORES = 8
VP2 = 30720        # padded full vocab (60 chunks of 512)
NVC = VP2 // 512
NPADC = float(VP2 - V)  # padded weight columns, each contributes exp(0)=1
NSAMP = 4            # sampled chunks (2048 cols) for the exp(t-1)-t piece
SSCL = float(V) / (NSAMP * 512)
LN_EPS = 1e-12
ALPHA = 0.01       # leaky relu slope
MASK_NEG = -5000.0  # pre-leaky masked logit; leaky -> ~-50 -> exp ~ 0
NKT = D // 128     # 6 hidden k-tiles for the GAT matmuls
NM = (B * N) // 128  # 16 token m-tiles
CW = 512           # phase-B moving width (PSUM bank limit for f32 out)
QW = 1920          # final stage width
LN2 = 0.6931471805599453
# ln(1+t) ~ sum a_k t^k on [0,1), max err 1.2e-5
LNC = [0.9994349429297625, -0.49134746165823384, 0.2878246937290064,
       -0.13413330582888625, 0.03137662229933151]

AX = mybir.AxisListType
AF = mybir.ActivationFunctionType
OP = mybir.AluOpType

_NC_CACHE = {}


def _build(with_ln_b: bool, with_out_b: bool):
    """Build the SPMD Bass program (identical on all 8 cores)."""
    nc = bacc.Bacc(
        "TRN2",
        target_bir_lowering=False,
        debug=False,
        enable_asserts=False,
        num_devices=NCORES,
    )

    # ---- per-core I/O --------------------------------------------------
    xpre = nc.dram_tensor("xpre", [N, D], F16, kind="ExternalInput").ap()
    mneg = nc.dram_tensor("mneg", [N, N], F16, kind="ExternalInput").ap()
    wmat = nc.dram_tensor("wmat", [D, H * F], F16, kind="ExternalInput").ap()
    wsum = nc.dram_tensor("wsum", [D, 2 * H], F16, kind="ExternalInput").ap()
    wst = nc.dram_tensor("wst", [4, 128, VP2], F16, kind="ExternalInput").ap()
    if with_ln_b:
        browm = nc.dram_tensor("browm", [1, H * F], F16, kind="ExternalInput").ap()
        brows = nc.dram_tensor("brows", [1, 2 * H], F16, kind="ExternalInput").ap()
    if with_out_b:
        bvoc = nc.dram_tensor("bvoc", [1, VP2], F16, kind="ExternalInput").ap()
    # f16 output staging: host converts to f32 (adds <0.008 abs err, budget
    # is ~0.46); halves the 31MB/core output DMA and doubles DVE throughput
    # on the final subtract.
    out = nc.dram_tensor("out", [N, VP2], F16, kind="ExternalOutput").ap()

    rg = [list(range(NCORES))]

    with tile.TileContext(nc) as tc:
        # ---- persistent SBUF ------------------------------------------
        with (
            tc.tile_pool(name="wpool", bufs=1) as wpool,
            tc.tile_pool(name="catf_pool", bufs=1) as catf_pool,
            tc.tile_pool(name="dram", bufs=1, space="DRAM") as dram,
        ):
            # cat (elu'd head features, [feat, tok]) persists into phase B
            cat_sb = [
                catf_pool.tile([128, N], F16, tag=f"cat{h}", name=f"cat{h}")
                for h in range(H)
            ]
            if with_out_b:
                bvoc_sb = wpool.tile([1, VP2], F16, tag="bvoc")
                ones1v = wpool.tile([1, 128], F16, tag="ones1v")
                nc.vector.memset(ones1v[:], 1.0)

            # ==== phase A: embedding LN + GAT (own batch) ==============
            with (
                tc.tile_pool(name="pa", bufs=1) as pa,
                tc.tile_pool(name="pa_tmp", bufs=2) as pa_tmp,
                tc.tile_pool(name="ps_a", bufs=1, space="PSUM") as ps_a,
            ):
                # input DMAs in latency order: LN input first, the big
                # vocab weights (not needed until phase B) last.
                xp_sb = [pa.tile([128, D], F16, tag=f"xp{m}", name=f"xp{m}") for m in range(2)]
                for m in range(2):
                    nc.sync.dma_start(
                        out=xp_sb[m][:, 0 : D // 2],
                        in_=xpre[m * 128 : (m + 1) * 128, 0 : D // 2],
                    )
                    nc.scalar.dma_start(
                        out=xp_sb[m][:, D // 2 : D],
                        in_=xpre[m * 128 : (m + 1) * 128, D // 2 : D],
                    )
                mneg_sb = [pa.tile([128, N], F16, tag=f"mneg{j}", name=f"mneg{j}") for j in range(2)]
                for jt in range(2):
                    nc.sync.dma_start(
                        out=mneg_sb[jt][:], in_=mneg[jt * 128 : (jt + 1) * 128, :]
                    )
                wmat_sb = [
                    pa.tile([128, H * F], F16, tag=f"wmat{kt}", name=f"wmat{kt}")
                    for kt in range(NKT)
                ]
                wsum_sb = [
                    pa.tile([128, 2 * H], F16, tag=f"wsum{kt}", name=f"wsum{kt}")
                    for kt in range(NKT)
                ]
                for kt in range(NKT):
                    nc.sync.dma_start(
                        out=wmat_sb[kt][:], in_=wmat[kt * 128 : (kt + 1) * 128, :]
                    )
                    nc.sync.dma_start(
                        out=wsum_sb[kt][:], in_=wsum[kt * 128 : (kt + 1) * 128, :]
                    )
                if with_ln_b:
                    browm_sb = pa.tile([1, H * F], F16, tag="browm")
                    nc.sync.dma_start(out=browm_sb[:], in_=browm[:, :])
                    brows_sb = pa.tile([1, 2 * H], F16, tag="brows")
                    nc.sync.dma_start(out=brows_sb[:], in_=brows[:, :])
                if with_out_b:
                    nc.sync.dma_start(out=bvoc_sb[:], in_=bvoc[:, :])

                idw = pa.tile([128, 128], F16, tag="idw")
                bass_masks_identity(nc, idw[:])
                ones1 = pa.tile([1, 128], F16, tag="ones1")
                nc.vector.memset(ones1[:], 1.0)
                eps_sb = pa.tile([128, 1], F32, tag="eps_sb")
                nc.vector.memset(eps_sb[:], LN_EPS)

                # ---- LayerNorm (tokens on partitions) -----------------
                # mean via vector reduce in parallel with sum-of-squares via
                # scalar Square+accum; then var = E[x^2]-mu^2 on tiny tiles.
                xn_sb = [pa.tile([128, D], F16, tag=f"xn{m}", name=f"xn{m}") for m in range(2)]
                for m in range(2):
                    xp = xp_sb[m]
                    xsum = pa_tmp.tile([128, 1], F32, tag="xsum")
                    nc.vector.tensor_reduce(
                        out=xsum[:], in_=xp[:], axis=AX.X, op=OP.add
                    )
                    sq = pa_tmp.tile([128, D], F32, tag="sq")
                    ssum = pa_tmp.tile([128, 1], F32, tag="ssum")
                    nc.scalar.activation(
                        sq[:], xp[:], AF.Square, accum_out=ssum[:, 0:1]
                    )
                    mu = pa_tmp.tile([128, 1], F32, tag="mu")
                    nc.vector.tensor_scalar_mul(mu[:], xsum[:], 1.0 / D)
                    ex2 = pa_tmp.tile([128, 1], F32, tag="ex2")
                    nc.vector.tensor_scalar_mul(ex2[:], ssum[:], 1.0 / D)
                    mu2 = pa_tmp.tile([128, 1], F32, tag="mu2")
                    nc.vector.tensor_scalar_mul(mu2[:], mu[:], mu[:, 0:1])
                    var = pa_tmp.tile([128, 1], F32, tag="var")
                    nc.vector.scalar_tensor_tensor(
                        var[:], ex2[:], 0.0, mu2[:], OP.add, OP.subtract
                    )
                    sd = pa_tmp.tile([128, 1], F32, tag="sd")
                    nc.scalar.activation(
                        sd[:], var[:], AF.Sqrt, bias=eps_sb[:, 0:1]
                    )
                    rstd = pa_tmp.tile([128, 1], F32, tag="rstd")
                    nc.vector.reciprocal(rstd[:], sd[:])
                    nc.vector.tensor_scalar(
                        xn_sb[m][:], xp[:], mu[:, 0:1], rstd[:, 0:1],
                        OP.subtract, OP.mult,
                    )

                # ---- transpose xn -> xT[kt] [128 hid, 256 tok] --------
                xt_sb = [pa.tile([128, N], F16, tag=f"xt{kt}", name=f"xt{kt}") for kt in range(NKT)]
                for kt in range(NKT):
                    for m in range(2):
                        ptr = ps_a.tile([128, 128], F16, tag="ptr", bufs=2)
                        nc.tensor.transpose(
                            ptr[:], xn_sb[m][:, kt * 128 : (kt + 1) * 128], idw[:]
                        )
                        nc.vector.tensor_scalar_mul(
                            xt_sb[kt][:, m * 128 : (m + 1) * 128], ptr[:], 1.0
                        )

                # ---- all-heads Wh GEMM + s1/s2 contractions -----------
                wh_all = [pa.tile([128, H * F], F16, tag=f"whall{m}", name=f"whall{m}") for m in range(2)]
                s12m = [pa.tile([128, 2 * H], F32, tag=f"s12m{m}", name=f"s12m{m}") for m in range(2)]
                s1r = [pa.tile([1, N], F16, tag=f"s1r{h}", name=f"s1r{h}") for h in range(H)]
                for m in range(2):
                    pwh = ps_a.tile([128, H * F], F32, tag="pwh", bufs=2)
                    for kt in range(NKT):
                        nc.tensor.matmul(
                            pwh[:],
                            xt_sb[kt][:, m * 128 : (m + 1) * 128],
                            wmat_sb[kt][:],
                            start=(kt == 0),
                            stop=(kt == NKT - 1) and not with_ln_b,
                        )
                    if with_ln_b:
                        nc.tensor.matmul(
                            pwh[:], ones1[:], browm_sb[:], start=False, stop=True
                        )
                    nc.vector.tensor_scalar_mul(wh_all[m][:], pwh[:], 1.0)

                    pws = ps_a.tile([128, 2 * H], F32, tag="pws", bufs=1)
                    for kt in range(NKT):
                        nc.tensor.matmul(
                            pws[:],
                            xt_sb[kt][:, m * 128 : (m + 1) * 128],
                            wsum_sb[kt][:],
                            start=(kt == 0),
                            stop=(kt == NKT - 1) and not with_ln_b,
                        )
                    if with_ln_b:
                        nc.tensor.matmul(
                            pws[:], ones1[:], brows_sb[:], start=False, stop=True
                        )
                    nc.vector.tensor_scalar_mul(s12m[m][:], pws[:], 1.0)
                    # per-head s1 column [128,1] -> row [1,128] via transpose
                    s12h = pa_tmp.tile([128, H], F16, tag="s12h")
                    nc.vector.tensor_scalar_mul(s12h[:], pws[:, 0:H], 1.0)
                    for h in range(H):
                        ps1h = ps_a.tile([1, 128], F16, tag="ps1h", bufs=1)
                        nc.tensor.transpose(ps1h[:], s12h[:, h : h + 1], idw[:])
                        nc.vector.tensor_scalar_mul(s1r[h][:, m * 128 : (m + 1) * 128], ps1h[:], 1.0)

                # ---- per-head attention + aggregation -----------------
                att = [
                    [pa.tile([128, N], F16, tag=f"att{h}_{m}", name=f"att{h}_{m}") for m in range(2)]
                    for h in range(H)
                ]

                for h in range(H):
                    # attention scores + column softmax (over i = free dim)
                    for jt in range(2):
                        # psum = broadcast(s1) + (-5000)*mask
                        pet = ps_a.tile([128, N], F32, tag="pet", bufs=2)
                        nc.tensor.matmul(
                            pet[:], ones1[:], s1r[h][:], start=True, stop=False
                        )
                        nc.tensor.matmul(
                            pet[:], idw[:], mneg_sb[jt][:], start=False, stop=True
                        )
                        et = pa_tmp.tile([128, N], F32, tag="et")
                        nc.vector.tensor_scalar_add(
                            et[:], pet[:], s12m[jt][:, H + h : H + h + 1]
                        )
                        lr = pa_tmp.tile([128, N], F32, tag="lr")
                        nc.vector.scalar_tensor_tensor(
                            lr[:], et[:], ALPHA, et[:], OP.mult, OP.max
                        )
                        # softmax without max-subtraction (f32 exp; max ~e19)
                        ex = pa_tmp.tile([128, N], F32, tag="ex")
                        asum = pa_tmp.tile([128, 1], F32, tag="asum")
                        nc.scalar.activation(
                            ex[:], lr[:], AF.Exp, accum_out=asum[:, 0:1]
                        )
                        rec = pa_tmp.tile([128, 1], F32, tag="rec")
                        nc.vector.reciprocal(rec[:], asum[:])
                        nc.vector.tensor_scalar_mul(
                            att[h][jt][:], ex[:], rec[:, 0:1]
                        )

                    # hp^T = Wh^T @ att^T, then elu -> catT rows of head h
                    php = ps_a.tile([128, N], F32, tag="pwh", bufs=2)
                    for jt in range(2):
                        nc.tensor.matmul(
                            php[:],
                            wh_all[jt][:, h * F : (h + 1) * F],
                            att[h][jt][:],
                            start=(jt == 0),
                            stop=(jt == 1),
                        )
                    e0h = pa_tmp.tile([128, N], F16, tag="e0h")
                    nc.scalar.activation(e0h[:], php[:], AF.Exp)
                    tmh = pa_tmp.tile([128, N], F16, tag="tmh")
                    nc.vector.tensor_scalar(
                        tmh[:], e0h[:], 1.0, -1.0, OP.min, OP.add
                    )
                    nc.vector.scalar_tensor_tensor(
                        cat_sb[h][:], php[:], 0.0, tmh[:], OP.max, OP.add
                    )

            # ==== full-vocab output linear + local log_softmax =========
            # batch-parallel: this core owns its 256 tokens for the FULL
            # vocab (no collectives). The 31.5MB weight streams in
            # 512-column chunks on the otherwise-idle gpsimd DMA queue.
            # Row sums without a second full exp pass: with q = elu(z)+1 =
            # max(z,0) + t, t = min(e^z,1),
            #   S = sum(e0) - NPAD + sum(exp(t-1) - t)
            # sum(e0) rides the exp-pass accumulators; (exp(t-1)-t) is
            # bounded in [0, 1/e] and estimated from a 2048-column sample,
            # so lnS is ready right after the last chunk and the finals
            # (q - 1 - lnS on the 4x DVE path) start immediately.
            with (
                tc.tile_pool(name="vp_pool", bufs=1) as vp_pool,
                tc.tile_pool(name="wstream", bufs=3) as wstream,
                tc.tile_pool(name="big_tmp", bufs=3) as big_tmp,
                tc.tile_pool(name="stat", bufs=1) as stat,
                tc.tile_pool(name="stage_pool", bufs=4) as stage_pool,
                tc.tile_pool(name="ps_z", bufs=8, space="PSUM") as ps_z,
            ):
                qt = [
                    vp_pool.tile([128, VP2], F16, tag=f"q{m}", name=f"q{m}")
                    for m in range(2)
                ]
                tsm = [
                    vp_pool.tile([128, NSAMP * 512], BF16, tag=f"ts{m}", name=f"ts{m}")
                    for m in range(2)
                ]
                eacc = stat.tile([128, NVC], F32, tag="eacc")
                ges = stat.tile([128, 2], F32, tag="ges")
                gts = stat.tile([128, 2], F32, tag="gts")
                bm1_sb = stat.tile([128, 1], F32, tag="bm1")
                nc.vector.memset(bm1_sb[:], -1.0)

                wv = None
                for vp in range(NVC // 2):
                    c0 = vp * 2 * CW
                    wva = wstream.tile(
                        [128, 4 * 2 * CW], F16, tag="wva", name="wva"
                    )
                    nc.gpsimd.dma_start(
                        out=wva[:].rearrange("p (k c) -> p k c", k=4),
                        in_=wst[:, :, c0 : c0 + 2 * CW].rearrange(
                            "k p c -> p k c"
                        ),
                    )

                    for m in range(2):
                        zp = ps_z.tile([128, 2 * CW], F32, tag="z", bufs=4)
                        for half in range(2):
                            hs = half * CW
                            for kt in range(4):
                                nc.tensor.matmul(
                                    zp[:, hs : hs + CW],
                                    cat_sb[kt][:, m * 128 : (m + 1) * 128],
                                    wva[:, kt * 2 * CW + hs : kt * 2 * CW + hs + CW],
                                    start=(kt == 0),
                                    stop=(kt == 3) and not with_out_b,
                                )
                            if with_out_b:
                                nc.tensor.matmul(
                                    zp[:, hs : hs + CW],
                                    ones1v[:],
                                    bvoc_sb[:, c0 + hs : c0 + hs + CW],
                                    start=False,
                                    stop=True,
                                )
                        e0 = big_tmp.tile([128, 2 * CW], BF16, tag="e0")
                        nc.scalar.activation(
                            e0[:], zp[:], AF.Exp,
                            accum_out=eacc[:, m * (NVC // 2) + vp : m * (NVC // 2) + vp + 1],
                        )
                        if vp < NSAMP // 2:
                            td = tsm[m][:, vp * 2 * CW : (vp + 1) * 2 * CW]
                        else:
                            tc_t = big_tmp.tile(
                                [128, 2 * CW], BF16, tag="tc", name="tc_t"
                            )
                            td = tc_t[:]
                        nc.vector.tensor_scalar_min(td, e0[:], 1.0)
                        nc.vector.scalar_tensor_tensor(
                            qt[m][:, c0 : c0 + 2 * CW], zp[:], 0.0, td,
                            OP.max, OP.add,
                        )

                # sampled sums of exp(t-1) and t (ACT; accumulators)
                for m in range(2):
                    es = big_tmp.tile([128, NSAMP * 512], F16, tag="es", name="es")
                    nc.scalar.activation(
                        es[:], tsm[m][:], AF.Exp, bias=bm1_sb[:, 0:1],
                        accum_out=ges[:, m : m + 1],
                    )
                    tsc = big_tmp.tile([128, NSAMP * 512], BF16, tag="tsc", name="tsc")
                    nc.scalar.activation(
                        tsc[:], tsm[m][:], AF.Copy,
                        accum_out=gts[:, m : m + 1],
                    )

                # S = sum(e0) - NPAD + SSCL*(sum(es) - sum(ts)) per token
                er = stat.tile([128, 2], F32, tag="er")
                nc.vector.tensor_reduce(
                    out=er[:],
                    in_=eacc[:].rearrange("p (m v) -> p m v", v=NVC // 2),
                    axis=AX.X, op=OP.add,
                )
                d1 = stat.tile([128, 2], F32, tag="d1")
                nc.vector.tensor_tensor(d1[:], ges[:], gts[:], OP.subtract)
                sp0 = stat.tile([128, 2], F32, tag="sp0")
                nc.vector.scalar_tensor_tensor(
                    sp0[:], d1[:], SSCL, er[:], OP.mult, OP.add
                )
                sg = stat.tile([128, 2], F32, tag="sg")
                nc.vector.tensor_scalar_add(sg[:], sp0[:], -NPADC)

                # negL = -ln(sg) - 1 via exponent bit-extract + poly
                ui = sg[:].bitcast(mybir.dt.uint32)
                ei = stat.tile([128, 2], mybir.dt.uint32, tag="ei")
                nc.vector.tensor_scalar(ei[:], ui, 23, None, OP.logical_shift_right)
                ef = stat.tile([128, 2], F32, tag="ef")
                nc.vector.tensor_scalar_mul(ef[:], ei[:], 1.0)
                mi = stat.tile([128, 2], mybir.dt.uint32, tag="mi")
                nc.vector.tensor_scalar(
                    mi[:], ui, 0x007FFFFF, 0x3F800000,
                    OP.bitwise_and, OP.bitwise_or,
                )
                tv = stat.tile([128, 2], F32, tag="tv")
                nc.vector.tensor_scalar_sub(tv[:], mi[:].bitcast(F32), 1.0)
                pz = stat.tile([128, 2], F32, tag="pz")
                nc.vector.tensor_scalar_mul(pz[:], tv[:], LNC[4])
                for aa in (LNC[3], LNC[2], LNC[1], LNC[0]):
                    nc.vector.scalar_tensor_tensor(
                        pz[:], pz[:], aa, tv[:], OP.add, OP.mult
                    )
                pzc = stat.tile([128, 2], F32, tag="pzc")
                nc.vector.tensor_scalar(
                    pzc[:], pz[:], -1.0, 127.0 * LN2 - 1.0, OP.mult, OP.add
                )
                negl = stat.tile([128, 2], F32, tag="negl")
                nc.vector.scalar_tensor_tensor(
                    negl[:], ef[:], -LN2, pzc[:], OP.mult, OP.add
                )

                # finals: out = q + negL (q = elu(z)+1), f16 4x path
                for m in range(2):
                    for oi in range(VP2 // QW):
                        c0 = oi * QW
                        stg = stage_pool.tile([128, QW], F16, tag="stg")
                        nc.vector.tensor_scalar_add(
                            stg[:], qt[m][:, c0 : c0 + QW], negl[:, m : m + 1]
                        )
                        nc.sync.dma_start(
                            out=out[m * 128 : (m + 1) * 128, c0 : c0 + QW],
                            in_=stg[:],
                        )

    nc.compile()
    return nc


def bass_masks_identity(nc, ident_ap):
    from concourse import masks

    masks.make_identity(nc, ident_ap)


def _host_prep(inputs):
    """Per-core input maps from full inputs (numpy only)."""
    tok = np.asarray(inputs["token_ids"])
    typ = np.asarray(inputs["type_ids"])
    syn = np.asarray(inputs["synset_ids"])
    hw = np.asarray(inputs["highway"]).astype(bool)
    tok_emb = np.asarray(inputs["tok_emb"], dtype=np.float32)
    type_emb = np.asarray(inputs["type_emb"], dtype=np.float32)
    pos_emb = np.asarray(inputs["pos_emb"], dtype=np.float32)
    ln_g = np.asarray(inputs["ln_g"], dtype=np.float32)
    ln_b = np.asarray(inputs["ln_b"], dtype=np.float32)
    W = np.asarray(inputs["W"], dtype=np.float32)
    a = np.asarray(inputs["a"], dtype=np.float32)
    out_W = np.asarray(inputs["out_W"], dtype=np.float32)
    out_b = np.asarray(inputs["out_b"], dtype=np.float32)

    # embeddings (host gather + add, f32 like the reference)
    x_pre = tok_emb[tok] + type_emb[typ] + pos_emb[:N][None]  # (B,N,D)

    # graph mask (host index logic), transposed to [j, i]
    vis = syn[:, :, None] == syn[:, None, :]
    s1m = (typ == 1) & hw
    s3m = (typ == 3) & hw
    d1 = np.isin(typ, [0, 2, 5]) & hw
    d3 = np.isin(typ, [6, 4, 0]) & hw
    vis = vis | (s1m[:, :, None] & d1[:, None, :]) | (s3m[:, :, None] & d3[:, None, :])
    mask = vis & (tok != 0)[:, None, :]  # (B,N,N) over [i,j]
    # -5000 where masked-out, 0 where visible; [j, i] layout
    mneg = np.where(mask.transpose(0, 2, 1), 0.0, MASK_NEG).astype(np.float16)

    # GAT weights: fold ln_g; separate Wh matrix and a1/a2 contractions
    Wg = W * ln_g[None, :, None]  # (H,D,F)
    a1, a2 = a[:, :F], a[:, F:]
    c1 = np.einsum("hdf,hf->hd", Wg, a1)  # (H,D)
    c2 = np.einsum("hdf,hf->hd", Wg, a2)
    wmat = Wg.transpose(1, 0, 2).reshape(D, H * F).astype(np.float16)
    wsum = np.concatenate([c1.T, c2.T], axis=1).astype(np.float16)  # (D, 2H)

    with_ln_b = bool(np.any(ln_b != 0.0))
    browm = brows = None
    if with_ln_b:
        b1 = np.einsum("hdf,hf->hd", W, a1)  # (H,D)
        b2 = np.einsum("hdf,hf->hd", W, a2)
        browm = np.einsum("d,hdf->hf", ln_b, W).reshape(1, H * F).astype(np.float16)
        brows = np.concatenate([b1 @ ln_b, b2 @ ln_b]).reshape(1, 2 * H).astype(
            np.float16
        )

    # full out_W^T (padded to 30720), identical on every core
    wpad = np.zeros((VP2, H * F), dtype=np.float32)
    wpad[:V] = out_W
    wst = np.ascontiguousarray(wpad.T.astype(np.float16).reshape(4, 128, VP2))
    with_out_b = bool(np.any(out_b != 0.0))
    bpad = np.zeros((VP2,), dtype=np.float32)
    bpad[:V] = out_b
    bvoc = bpad.reshape(1, VP2).astype(np.float16)

    in_maps = []
    for c in range(NCORES):
        m = {
            "xpre": np.ascontiguousarray(x_pre[c]).astype(np.float16),
            "mneg": np.ascontiguousarray(mneg[c]),
            "wmat": wmat,
            "wsum": wsum,
            "wst": wst,
        }
        if with_ln_b:
            m["browm"] = browm
            m["brows"] = brows
        if with_out_b:
            m["bvoc"] = bvoc
        in_maps.append(m)
    return in_maps, with_ln_b, with_out_b


def kernel(**inputs) -> np.ndarray:
    in_maps, with_ln_b, with_out_b = _host_prep(inputs)

    key = (with_ln_b, with_out_b)
    if key not in _NC_CACHE:
        _NC_CACHE[key] = _build(with_ln_b, with_out_b)
    nc = _NC_CACHE[key]

    trace = bool(int(os.environ.get("KBERT_TRACE", "0")))
    res = run_bass_kernel_spmd(
        nc, in_maps, core_ids=list(range(NCORES)), trace=trace
    )
    if trace and res.exec_time_ns is not None:
        print(f"HW exec time: {res.exec_time_ns} ns")
        if res.instructions_and_trace is not None:
            print(f"trace: {res.instructions_and_trace[1]}")

    full = np.empty((B * N, V), dtype=np.float32)
    for c in range(NCORES):
        full[c * N : (c + 1) * N, :] = res.results[c]["out"][:, :V]
    return np.ascontiguousarray(full.reshape(B, N, V))



# revision 35
# speedup vs baseline: 1.1411x; 1.1411x over previous
"""KBertGATEnricher Trainium2 kernel.

Sharding: data-parallel over batch (8 batches -> 8 cores) for the whole
model; each core computes its 256 tokens against the FULL vocab, so there
are no collectives at all. The 31.5MB f16 vocab weight streams in
1024-column chunk pairs on the gpsimd DMA queue while the PE runs the
output GEMM out of 8 PSUM banks.

log_softmax without a second full exp pass: with q = elu(z)+1 =
max(z,0) + t and t = min(e^z,1),

    S = sum(e^z) - n_pad + sum(exp(t-1) - t)

sum(e^z) rides the f32 accumulators of the exp pass we need anyway for
elu (bf16 output avoids both f16 overflow at z~21 and f16-subnormal
flushing near z~0); exp(t-1)-t is bounded in [0, 1/e] and vanishes for
z>=0, so its sum is estimated from a 2048-column sample (~0.3% error on S
against a 2e-2 budget). lnS is therefore ready ~2us after the last GEMM
chunk and the tail is just the final q-1-lnS pass (4x-mode DVE) overlapped
with the 15.7MB output DMA.

Self-contained: hardcodes all shapes; only imports the system-installed
concourse runtime.
"""

import os
import sys

sys.path.insert(0, "/opt/trn_rl_repo")

import numpy as np

from concourse import bass, bacc, mybir, tile
from concourse.bass_utils import run_bass_kernel_spmd

F32 = mybir.dt.float32
F16 = mybir.dt.float16
BF16 = mybir.dt.bfloat16

B, N, D, H, F, V = 8, 256, 768, 4, 128, 30522
NCORES = 8
VP2 = 30720        # padded full vocab (60 chunks of 512)
NVC = VP2 // 512
NPADC = float(VP2 - V)  # padded weight columns, each contributes exp(0)=1
NSAMP = 4            # sampled chunks (2048 cols) for the exp(t-1)-t piece
SSCL = float(V) / (NSAMP * 512)
LN_EPS = 1e-12
ALPHA = 0.01       # leaky relu slope
MASK_NEG = -5000.0  # pre-leaky masked logit; leaky -> ~-50 -> exp ~ 0
NKT = D // 128     # 6 hidden k-tiles for the GAT matmuls
NM = (B * N) // 128  # 16 token m-tiles
CW = 512           # phase-B moving width (PSUM bank limit for f32 out)
QW = 1920          # final stage width
LN2 = 0.6931471805599453
# ln(1+t) ~ sum a_k t^k on [0,1), max err 1.2e-5
LNC = [0.9994349429297625, -0.49134746165823384, 0.2878246937290064,
       -0.13413330582888625, 0.03137662229933151]

AX = mybir.AxisListType
AF = mybir.ActivationFunctionType
OP = mybir.AluOpType

_NC_CACHE = {}


def _build(with_ln_b: bool, with_out_b: bool):
    """Build the SPMD Bass program (identical on all 8 cores)."""
    nc = bacc.Bacc(
        "TRN2",
        target_bir_lowering=False,
        debug=False,
        enable_asserts=False,
        num_devices=NCORES,
    )

    # ---- per-core I/O --------------------------------------------------
    xpre = nc.dram_tensor("xpre", [N, D], F16, kind="ExternalInput").ap()
    mneg = nc.dram_tensor("mneg", [N, N], F16, kind="ExternalInput").ap()
    wmat = nc.dram_tensor("wmat", [D, H * F], F16, kind="ExternalInput").ap()
    wsum = nc.dram_tensor("wsum", [D, 2 * H], F16, kind="ExternalInput").ap()
    wst = nc.dram_tensor("wst", [4, 128, VP2], F16, kind="ExternalInput").ap()
    if with_ln_b:
        browm = nc.dram_tensor("browm", [1, H * F], F16, kind="ExternalInput").ap()
        brows = nc.dram_tensor("brows", [1, 2 * H], F16, kind="ExternalInput").ap()
    if with_out_b:
        bvoc = nc.dram_tensor("bvoc", [1, VP2], F16, kind="ExternalInput").ap()
    # f16 output staging: host converts to f32 (adds <0.008 abs err, budget
    # is ~0.46); halves the 31MB/core output DMA and doubles DVE throughput
    # on the final subtract.
    out = nc.dram_tensor("out", [N, VP2], F16, kind="ExternalOutput").ap()

    rg = [list(range(NCORES))]

    with tile.TileContext(nc) as tc:
        # ---- persistent SBUF ------------------------------------------
        with (
            tc.tile_pool(name="wpool", bufs=1) as wpool,
            tc.tile_pool(name="catf_pool", bufs=1) as catf_pool,
            tc.tile_pool(name="dram", bufs=1, space="DRAM") as dram,
        ):
            # cat (elu'd head features, [feat, tok]) persists into phase B
            cat_sb = [
                catf_pool.tile([128, N], F16, tag=f"cat{h}", name=f"cat{h}")
                for h in range(H)
            ]
            if with_out_b:
                bvoc_sb = wpool.tile([1, VP2], F16, tag="bvoc")
                ones1v = wpool.tile([1, 128], F16, tag="ones1v")
                nc.vector.memset(ones1v[:], 1.0)

            # ==== phase A: embedding LN + GAT (own batch) ==============
            with (
                tc.tile_pool(name="pa", bufs=1) as pa,
                tc.tile_pool(name="pa_tmp", bufs=2) as pa_tmp,
                tc.tile_pool(name="ps_a", bufs=1, space="PSUM") as ps_a,
            ):
                # input DMAs in latency order: LN input first, the big
                # vocab weights (not needed until phase B) last.
                xp_sb = [pa.tile([128, D], F16, tag=f"xp{m}", name=f"xp{m}") for m in range(2)]
                for m in range(2):
                    nc.sync.dma_start(
                        out=xp_sb[m][:, 0 : D // 2],
                        in_=xpre[m * 128 : (m + 1) * 128, 0 : D // 2],
                    )
                    nc.scalar.dma_start(
                        out=xp_sb[m][:, D // 2 : D],
                        in_=xpre[m * 128 : (m + 1) * 128, D // 2 : D],
                    )
                mneg_sb = [pa.tile([128, N], F16, tag=f"mneg{j}", name=f"mneg{j}") for j in range(2)]
                for jt in range(2):
                    nc.sync.dma_start(
                        out=mneg_sb[jt][:], in_=mneg[jt * 128 : (jt + 1) * 128, :]
                    )
                wmat_sb = [
                    pa.tile([128, H * F], F16, tag=f"wmat{kt}", name=f"wmat{kt}")
                    for kt in range(NKT)
                ]
                wsum_sb = [
                    pa.tile([128, 2 * H], F16, tag=f"wsum{kt}", name=f"wsum{kt}")
                    for kt in range(NKT)
                ]
                for kt in range(NKT):
                    nc.sync.dma_start(
                        out=wmat_sb[kt][:], in_=wmat[kt * 128 : (kt + 1) * 128, :]
                    )
                    nc.sync.dma_start(
                        out=wsum_sb[kt][:], in_=wsum[kt * 128 : (kt + 1) * 128, :]
                    )
                if with_ln_b:
                    browm_sb = pa.tile([1, H * F], F16, tag="browm")
                    nc.sync.dma_start(out=browm_sb[:], in_=browm[:, :])
                    brows_sb = pa.tile([1, 2 * H], F16, tag="brows")
                    nc.sync.dma_start(out=brows_sb[:], in_=brows[:, :])
                if with_out_b:
                    nc.sync.dma_start(out=bvoc_sb[:], in_=bvoc[:, :])

                idw = pa.tile([128, 128], F16, tag="idw")
                bass_masks_identity(nc, idw[:])
                ones1 = pa.tile([1, 128], F16, tag="ones1")
                nc.vector.memset(ones1[:], 1.0)
                eps_sb = pa.tile([128, 1], F32, tag="eps_sb")
                nc.vector.memset(eps_sb[:], LN_EPS)

                # ---- LayerNorm (tokens on partitions) -----------------
                # mean via vector reduce in parallel with sum-of-squares via
                # scalar Square+accum; then var = E[x^2]-mu^2 on tiny tiles.
                xn_sb = [pa.tile([128, D], F16, tag=f"xn{m}", name=f"xn{m}") for m in range(2)]
                for m in range(2):
                    xp = xp_sb[m]
                    xsum = pa_tmp.tile([128, 1], F32, tag="xsum")
                    nc.vector.tensor_reduce(
                        out=xsum[:], in_=xp[:], axis=AX.X, op=OP.add
                    )
                    sq = pa_tmp.tile([128, D], F32, tag="sq")
                    ssum = pa_tmp.tile([128, 1], F32, tag="ssum")
                    nc.scalar.activation(
                        sq[:], xp[:], AF.Square, accum_out=ssum[:, 0:1]
                    )
                    mu = pa_tmp.tile([128, 1], F32, tag="mu")
                    nc.vector.tensor_scalar_mul(mu[:], xsum[:], 1.0 / D)
                    ex2 = pa_tmp.tile([128, 1], F32, tag="ex2")
                    nc.vector.tensor_scalar_mul(ex2[:], ssum[:], 1.0 / D)
                    mu2 = pa_tmp.tile([128, 1], F32, tag="mu2")
                    nc.vector.tensor_scalar_mul(mu2[:], mu[:], mu[:, 0:1])
                    var = pa_tmp.tile([128, 1], F32, tag="var")
                    nc.vector.scalar_tensor_tensor(
                        var[:], ex2[:], 0.0, mu2[:], OP.add, OP.subtract
                    )
                    sd = pa_tmp.tile([128, 1], F32, tag="sd")
                    nc.scalar.activation(
                        sd[:], var[:], AF.Sqrt, bias=eps_sb[:, 0:1]
                    )
                    rstd = pa_tmp.tile([128, 1], F32, tag="rstd")
                    nc.vector.reciprocal(rstd[:], sd[:])
                    nc.vector.tensor_scalar(
                        xn_sb[m][:], xp[:], mu[:, 0:1], rstd[:, 0:1],
                        OP.subtract, OP.mult,
                    )

                # ---- transpose xn -> xT[kt] [128 hid, 256 tok] --------
                xt_sb = [pa.tile([128, N], F16, tag=f"xt{kt}", name=f"xt{kt}") for kt in range(NKT)]
                for kt in range(NKT):
                    for m in range(2):
                        ptr = ps_a.tile([128, 128], F16, tag="ptr", bufs=2)
                        nc.tensor.transpose(
                            ptr[:], xn_sb[m][:, kt * 128 : (kt + 1) * 128], idw[:]
                        )
                        nc.vector.tensor_scalar_mul(
                            xt_sb[kt][:, m * 128 : (m + 1) * 128], ptr[:], 1.0
                        )

                # ---- all-heads Wh GEMM + s1/s2 contractions -----------
                wh_all = [pa.tile([128, H * F], F16, tag=f"whall{m}", name=f"whall{m}") for m in range(2)]
                s12m = [pa.tile([128, 2 * H], F32, tag=f"s12m{m}", name=f"s12m{m}") for m in range(2)]
                s1r = [pa.tile([1, N], F16, tag=f"s1r{h}", name=f"s1r{h}") for h in range(H)]
                for m in range(2):
                    pwh = ps_a.tile([128, H * F], F32, tag="pwh", bufs=2)
                    for kt in range(NKT):
                        nc.tensor.matmul(
                            pwh[:],
                            xt_sb[kt][:, m * 128 : (m + 1) * 128],
                            wmat_sb[kt][:],
                            start=(kt == 0),
                            stop=(kt == NKT - 1) and not with_ln_b,
                        )
                    if with_ln_b:
                        nc.tensor.matmul(
                            pwh[:], ones1[:], browm_sb[:], start=False, stop=True
                        )
                    nc.vector.tensor_scalar_mul(wh_all[m][:], pwh[:], 1.0)

                    pws = ps_a.tile([128, 2 * H], F32, tag="pws", bufs=1)
                    for kt in range(NKT):
                        nc.tensor.matmul(
                            pws[:],
                            xt_sb[kt][:, m * 128 : (m + 1) * 128],
                            wsum_sb[kt][:],
                            start=(kt == 0),
                            stop=(kt == NKT - 1) and not with_ln_b,
                        )
                    if with_ln_b:
                        nc.tensor.matmul(
                            pws[:], ones1[:], brows_sb[:], start=False, stop=True
                        )
                    nc.vector.tensor_scalar_mul(s12m[m][:], pws[:], 1.0)
                    # per-head s1 column [128,1] -> row [1,128] via transpose
                    s12h = pa_tmp.tile([128, H], F16, tag="s12h")
                    nc.vector.tensor_scalar_mul(s12h[:], pws[:, 0:H], 1.0)
                    for h in range(H):
                        ps1h = ps_a.tile([1, 128], F16, tag="ps1h", bufs=1)
                        nc.tensor.transpose(ps1h[:], s12h[:, h : h + 1], idw[:])
                        nc.vector.tensor_scalar_mul(s1r[h][:, m * 128 : (m + 1) * 128], ps1h[:], 1.0)

                # ---- per-head attention + aggregation -----------------
                att = [
                    [pa.tile([128, N], F16, tag=f"att{h}_{m}", name=f"att{h}_{m}") for m in range(2)]
                    for h in range(H)
                ]

                for h in range(H):
                    # attention scores + column softmax (over i = free dim)
                    for jt in range(2):
                        # psum = broadcast(s1) + (-5000)*mask
                        pet = ps_a.tile([128, N], F32, tag="pet", bufs=2)
                        nc.tensor.matmul(
                            pet[:], ones1[:], s1r[h][:], start=True, stop=False
                        )
                        nc.tensor.matmul(
                            pet[:], idw[:], mneg_sb[jt][:], start=False, stop=True
                        )
                        et = pa_tmp.tile([128, N], F32, tag="et")
                        nc.vector.tensor_scalar_add(
                            et[:], pet[:], s12m[jt][:, H + h : H + h + 1]
                        )
                        lr = pa_tmp.tile([128, N], F32, tag="lr")
                        nc.vector.scalar_tensor_tensor(
                            lr[:], et[:], ALPHA, et[:], OP.mult, OP.max
                        )
                        # softmax without max-subtraction (f32 exp; max ~e19)
                        ex = pa_tmp.tile([128, N], F32, tag="ex")
                        asum = pa_tmp.tile([128, 1], F32, tag="asum")
                        nc.scalar.activation(
                            ex[:], lr[:], AF.Exp, accum_out=asum[:, 0:1]
                        )
                        rec = pa_tmp.tile([128, 1], F32, tag="rec")
                        nc.vector.reciprocal(rec[:], asum[:])
                        nc.vector.tensor_scalar_mul(
                            att[h][jt][:], ex[:], rec[:, 0:1]
                        )

                    # hp^T = Wh^T @ att^T, then elu -> catT rows of head h
                    php = ps_a.tile([128, N], F32, tag="pwh", bufs=2)
                    for jt in range(2):
                        nc.tensor.matmul(
                            php[:],
                            wh_all[jt][:, h * F : (h + 1) * F],
                            att[h][jt][:],
                            start=(jt == 0),
                            stop=(jt == 1),
                        )
                    e0h = pa_tmp.tile([128, N], F16, tag="e0h")
                    nc.scalar.activation(e0h[:], php[:], AF.Exp)
                    tmh = pa_tmp.tile([128, N], F16, tag="tmh")
                    nc.vector.tensor_scalar(
                        tmh[:], e0h[:], 1.0, -1.0, OP.min, OP.add
                    )
                    nc.vector.scalar_tensor_tensor(
                        cat_sb[h][:], php[:], 0.0, tmh[:], OP.max, OP.add
                    )

            # ==== full-vocab output linear + local log_softmax =========
            # batch-parallel: this core owns its 256 tokens for the FULL
            # vocab (no collectives). The 31.5MB weight streams in
            # 512-column chunks on the otherwise-idle gpsimd DMA queue.
            # Row sums without a second full exp pass: with q = elu(z)+1 =
            # max(z,0) + t, t = min(e^z,1),
            #   S = sum(e0) - NPAD + sum(exp(t-1) - t)
            # sum(e0) rides the exp-pass accumulators; (exp(t-1)-t) is
            # bounded in [0, 1/e] and estimated from a 2048-column sample,
            # so lnS is ready right after the last chunk and the finals
            # (q - 1 - lnS on the 4x DVE path) start immediately.
            with (
                tc.tile_pool(name="vp_pool", bufs=1) as vp_pool,
                tc.tile_pool(name="wstream", bufs=3) as wstream,
                tc.tile_pool(name="big_tmp", bufs=3) as big_tmp,
                tc.tile_pool(name="stat", bufs=1) as stat,
                tc.tile_pool(name="stage_pool", bufs=4) as stage_pool,
                tc.tile_pool(name="ps_z", bufs=8, space="PSUM") as ps_z,
            ):
                qt = [
                    vp_pool.tile([128, VP2], F16, tag=f"q{m}", name=f"q{m}")
                    for m in range(2)
                ]
                tsm = [
                    vp_pool.tile([128, NSAMP * 512], BF16, tag=f"ts{m}", name=f"ts{m}")
                    for m in range(2)
                ]
                eacc = stat.tile([128, NVC], F32, tag="eacc")
                ges = stat.tile([128, 2], F32, tag="ges")
                gts = stat.tile([128, 2], F32, tag="gts")
                bm1_sb = stat.tile([128, 1], F32, tag="bm1")
                nc.vector.memset(bm1_sb[:], -1.0)

                wv = None
                for vp in range(NVC // 2):
                    c0 = vp * 2 * CW
                    wv = [
                        wstream.tile(
                            [128, 2 * CW], F16, tag=f"wv{kt}", name=f"wv{kt}"
                        )
                        for kt in range(4)
                    ]
                    for kt in range(4):
                        nc.gpsimd.dma_start(
                            out=wv[kt][:], in_=wst[kt, :, c0 : c0 + 2 * CW]
                        )

                    for m in range(2):
                        zp = ps_z.tile([128, 2 * CW], F32, tag="z", bufs=4)
                        for half in range(2):
                            hs = half * CW
                            for kt in range(4):
                                nc.tensor.matmul(
                                    zp[:, hs : hs + CW],
                                    cat_sb[kt][:, m * 128 : (m + 1) * 128],
                                    wv[kt][:, hs : hs + CW],
                                    start=(kt == 0),
                                    stop=(kt == 3) and not with_out_b,
                                )
                            if with_out_b:
                                nc.tensor.matmul(
                                    zp[:, hs : hs + CW],
                                    ones1v[:],
                                    bvoc_sb[:, c0 + hs : c0 + hs + CW],
                                    start=False,
                                    stop=True,
                                )
                        e0 = big_tmp.tile([128, 2 * CW], BF16, tag="e0")
                        nc.scalar.activation(
                            e0[:], zp[:], AF.Exp,
                            accum_out=eacc[:, m * (NVC // 2) + vp : m * (NVC // 2) + vp + 1],
                        )
                        if vp < NSAMP // 2:
                            td = tsm[m][:, vp * 2 * CW : (vp + 1) * 2 * CW]
                        else:
                            tc_t = big_tmp.tile(
                                [128, 2 * CW], BF16, tag="tc", name="tc_t"
                            )
                            td = tc_t[:]
                        nc.vector.tensor_scalar_min(td, e0[:], 1.0)
                        nc.vector.scalar_tensor_tensor(
                            qt[m][:, c0 : c0 + 2 * CW], zp[:], 0.0, td,
                            OP.max, OP.add,
                        )

                # sampled sums of exp(t-1) and t (ACT; accumulators)
                for m in range(2):
                    es = big_tmp.tile([128, NSAMP * 512], F16, tag="es", name="es")
                    nc.scalar.activation(
                        es[:], tsm[m][:], AF.Exp, bias=bm1_sb[:, 0:1],
                        accum_out=ges[:, m : m + 1],
                    )
                    tsc = big_tmp.tile([128, NSAMP * 512], BF16, tag="tsc", name="tsc")
                    nc.scalar.activation(
                        tsc[:], tsm[m][:], AF.Copy,
                        accum_out=gts[:, m : m + 1],
                    )

                # S = sum(e0) - NPAD + SSCL*(sum(es) - sum(ts)) per token
                er = stat.tile([128, 2], F32, tag="er")
                nc.vector.tensor_reduce(
                    out=er[:],
                    in_=eacc[:].rearrange("p (m v) -> p m v", v=NVC // 2),
                    axis=AX.X, op=OP.add,
                )
                d1 = stat.tile([128, 2], F32, tag="d1")
                nc.vector.tensor_tensor(d1[:], ges[:], gts[:], OP.subtract)
                sp0 = stat.tile([128, 2], F32, tag="sp0")
                nc.vector.scalar_tensor_tensor(
                    sp0[:], d1[:], SSCL, er[:], OP.mult, OP.add
                )
                sg = stat.tile([128, 2], F32, tag="sg")
                nc.vector.tensor_scalar_add(sg[:], sp0[:], -NPADC)

                # negL = -ln(sg) - 1 via exponent bit-extract + poly
                ui = sg[:].bitcast(mybir.dt.uint32)
                ei = stat.tile([128, 2], mybir.dt.uint32, tag="ei")
                nc.vector.tensor_scalar(ei[:], ui, 23, None, OP.logical_shift_right)
                ef = stat.tile([128, 2], F32, tag="ef")
                nc.vector.tensor_scalar_mul(ef[:], ei[:], 1.0)
                mi = stat.tile([128, 2], mybir.dt.uint32, tag="mi")
                nc.vector.tensor_scalar(
                    mi[:], ui, 0x007FFFFF, 0x3F800000,
                    OP.bitwise_and, OP.bitwise_or,
                )
                tv = stat.tile([128, 2], F32, tag="tv")
                nc.vector.tensor_scalar_sub(tv[:], mi[:].bitcast(F32), 1.0)
                pz = stat.tile([128, 2], F32, tag="pz")
                nc.vector.tensor_scalar_mul(pz[:], tv[:], LNC[4])
                for aa in (LNC[3], LNC[2], LNC[1], LNC[0]):
                    nc.vector.scalar_tensor_tensor(
                        pz[:], pz[:], aa, tv[:], OP.add, OP.mult
                    )
                pzc = stat.tile([128, 2], F32, tag="pzc")
                nc.vector.tensor_scalar(
                    pzc[:], pz[:], -1.0, 127.0 * LN2 - 1.0, OP.mult, OP.add
                )
                negl = stat.tile([128, 2], F32, tag="negl")
                nc.vector.scalar_tensor_tensor(
                    negl[:], ef[:], -LN2, pzc[:], OP.mult, OP.add
                )

                # finals: out = q + negL (q = elu(z)+1), f16 4x path
                for m in range(2):
                    for oi in range(VP2 // QW):
                        c0 = oi * QW
                        stg = stage_pool.tile([128, QW], F16, tag="stg")
                        nc.vector.tensor_scalar_add(
                            stg[:], qt[m][:, c0 : c0 + QW], negl[:, m : m + 1]
                        )
                        nc.sync.dma_start(
                            out=out[m * 128 : (m + 1) * 128, c0 : c0 + QW],
                            in_=stg[:],
                        )

    nc.compile()
    return nc


def bass_masks_identity(nc, ident_ap):
    from concourse import masks

    masks.make_identity(nc, ident_ap)


def _host_prep(inputs):
    """Per-core input maps from full inputs (numpy only)."""
    tok = np.asarray(inputs["token_ids"])
    typ = np.asarray(inputs["type_ids"])
    syn = np.asarray(inputs["synset_ids"])
    hw = np.asarray(inputs["highway"]).astype(bool)
    tok_emb = np.asarray(inputs["tok_emb"], dtype=np.float32)
    type_emb = np.asarray(inputs["type_emb"], dtype=np.float32)
    pos_emb = np.asarray(inputs["pos_emb"], dtype=np.float32)
    ln_g = np.asarray(inputs["ln_g"], dtype=np.float32)
    ln_b = np.asarray(inputs["ln_b"], dtype=np.float32)
    W = np.asarray(inputs["W"], dtype=np.float32)
    a = np.asarray(inputs["a"], dtype=np.float32)
    out_W = np.asarray(inputs["out_W"], dtype=np.float32)
    out_b = np.asarray(inputs["out_b"], dtype=np.float32)

    # embeddings (host gather + add, f32 like the reference)
    x_pre = tok_emb[tok] + type_emb[typ] + pos_emb[:N][None]  # (B,N,D)

    # graph mask (host index logic), transposed to [j, i]
    vis = syn[:, :, None] == syn[:, None, :]
    s1m = (typ == 1) & hw
    s3m = (typ == 3) & hw
    d1 = np.isin(typ, [0, 2, 5]) & hw
    d3 = np.isin(typ, [6, 4, 0]) & hw
    vis = vis | (s1m[:, :, None] & d1[:, None, :]) | (s3m[:, :, None] & d3[:, None, :])
    mask = vis & (tok != 0)[:, None, :]  # (B,N,N) over [i,j]
    # -5000 where masked-out, 0 where visible; [j, i] layout
    mneg = np.where(mask.transpose(0, 2, 1), 0.0, MASK_NEG).astype(np.float16)

    # GAT weights: fold ln_g; separate Wh matrix and a1/a2 contractions
    Wg = W * ln_g[None, :, None]  # (H,D,F)
    a1, a2 = a[:, :F], a[:, F:]
    c1 = np.einsum("hdf,hf->hd", Wg, a1)  # (H,D)
    c2 = np.einsum("hdf,hf->hd", Wg, a2)
    wmat = Wg.transpose(1, 0, 2).reshape(D, H * F).astype(np.float16)
    wsum = np.concatenate([c1.T, c2.T], axis=1).astype(np.float16)  # (D, 2H)

    with_ln_b = bool(np.any(ln_b != 0.0))
    browm = brows = None
    if with_ln_b:
        b1 = np.einsum("hdf,hf->hd", W, a1)  # (H,D)
        b2 = np.einsum("hdf,hf->hd", W, a2)
        browm = np.einsum("d,hdf->hf", ln_b, W).reshape(1, H * F).astype(np.float16)
        brows = np.concatenate([b1 @ ln_b, b2 @ ln_b]).reshape(1, 2 * H).astype(
            np.float16
        )

    # full out_W^T (padded to 30720), identical on every core
    wpad = np.zeros((VP2, H * F), dtype=np.float32)
    wpad[:V] = out_W
    wst = np.ascontiguousarray(wpad.T.astype(np.float16).reshape(4, 128, VP2))
    with_out_b = bool(np.any(out_b != 0.0))
    bpad = np.zeros((VP2,), dtype=np.float32)
    bpad[:V] = out_b
    bvoc = bpad.reshape(1, VP2).astype(np.float16)

    in_maps = []
    for c in range(NCORES):
        m = {
            "xpre": np.ascontiguousarray(x_pre[c]).astype(np.float16),
            "mneg": np.ascontiguousarray(mneg[c]),
            "wmat": wmat,
            "wsum": wsum,
            "wst": wst,
        }
        if with_ln_b:
            m["browm"] = browm
            m["brows"] = brows
        if with_out_b:
            m["bvoc"] = bvoc
        in_maps.append(m)
    return in_maps, with_ln_b, with_out_b


def kernel(**inputs) -> np.ndarray:
    in_maps, with_ln_b, with_out_b = _host_prep(inputs)

    key = (with_ln_b, with_out_b)
    if key not in _NC_CACHE:
        _NC_CACHE[key] = _build(with_ln_b, with_out_b)
    nc = _NC_CACHE[key]

    trace = bool(int(os.environ.get("KBERT_TRACE", "0")))
    res = run_bass_kernel_spmd(
        nc, in_maps, core_ids=list(range(NCORES)), trace=trace
    )
    if trace and res.exec_time_ns is not None:
        print(f"HW exec time: {res.exec_time_ns} ns")
        if res.instructions_and_trace is not None:
            print(f"trace: {res.instructions_and_trace[1]}")

    full = np.empty((B * N, V), dtype=np.float32)
    for c in range(NCORES):
        full[c * N : (c + 1) * N, :] = res.results[c]["out"][:, :V]
    return np.ascontiguousarray(full.reshape(B, N, V))

